# revision 1
# baseline (speedup 1.0000x reference)
"""MoE layer (B=4, T=2048, C=1024, F=4096, E=8, top-2) on 8 trn2 NeuronCores.

Strategy: 8-way tensor parallelism over the FFN width F (not expert
parallelism).  The gate + top-2 routing runs on the host; tokens are
gathered into per-expert segments (total Sum n_e = N*topk = 16384
token-expert pairs).  EVERY core processes ALL 16384 pairs, but only a
1/8 slice of F (F_local = 512) of every expert:

    layer1:  h_loc = gelu(x @ w1[e][:, c*512:(c+1)*512] + b1_loc)
    layer2:  y_part = h_loc @ w2[e][c*512:(c+1)*512, :]      (no bias)

The host sums the 8 partial y's, adds b2, applies the top-2 combine
weights and scatter-adds into the full output.  Because all cores run
the exact same token segments, the SPMD program is naturally
shape-uniform with ZERO padding: per-core work = 16384 * 64 PE cycles
= 437 us at the 78.6 TF/s bf16 roofline, independent of expert load
imbalance.

Perf-critical structure (from trace analysis):
 - Every dma_start costs ~625 ns on the shared HWDGE descriptor
   generator regardless of size, so DMAs are BATCHED: one 3D-AP DMA
   per token tile / output tile / half-expert weight chunk (~120 DMAs
   total).  DRAM layouts are partition-major ([128, ...]) so a single
   DMA matches the SBUF tile layout.
 - Weights stream just-in-time in ~0.5 MB half-chunks spread across
   the previous expert's tiles, each emitted AFTER that tile's xt
   trigger, so token loads are never queued behind a weight transfer
   (the sync-engine DMA stream is in-order and self-paced by pool-
   buffer WAR dependencies).
 - y output DMAs ride the scalar engine's queue, decoupled from the
   sync stream.  (gpsimd DMA would decouple too, but Q7 activity
   downclocks the PE ~20% — measured.)
 - Software pipelining: layer1 of tile t+1 is emitted before layer2 of
   tile t, hiding the ~600 ns GELU latency of the last h-block (layer2
   only has a 4-deep contraction here, too short to hide it alone).
 - Two small starter tiles (256 tokens) + kc-half-split of tile 0's
   xt/w1 loads start the PE ~2 MB into the DMA stream; the final tile
   drains per-mc on the idle sync engine to trim the exec tail.

Device layout (weights stationary, tokens stream as moving operand):
  xt   [128, KC, NTOK]      bf16  all routed tokens, transposed
  w1l  [128, E, KC, FL]     bf16  w1[e][kc-slice, local F cols]
  b1l  [128, E*KFL]         f32   local b1 (partition = F%128)
  w2l  [128, E, KFL, C]     bf16  w2[e][local F rows, :]
  yt   [128, KC, NTOK]      bf16  partial y, transposed
"""

import numpy as np
import ml_dtypes

B, T, C, F, E, TOPK = 4, 2048, 1024, 4096, 8, 2
N_CORES = 8
KC = C // 128          # 8  C-slices (layer-1 contraction / layer-2 output)
FL = F // N_CORES      # 512 local F columns per core
KFL = FL // 128        # 4  local F-slices
TOK_TILE = 512
TILE0 = 256            # small starter tiles while the DMA stream ramps

_BF16 = ml_dtypes.bfloat16

_nc_cache: dict[tuple, object] = {}


def _token_tiles(cap: int, first_small: bool, last_small: bool = False):
    """Split cap into equal-ish tiles of at most TOK_TILE tokens.

    Equal sizes keep every matmul's streaming time above the LDWEIGHTS
    shadow (a small tail tile would be weight-load-bound on the PE;
    256 tokens still clears the ~233-cycle shadow).  Expert 0 starts
    with two 256-token tiles (enough compute to cover the DMA ramp,
    small enough to start the PE early); the last expert ends with one
    256-token tile so the final cast->DMA drain after the last matmul
    is short."""
    tiles = []
    off = 0
    if first_small and cap > 4 * TILE0:
        # starters (256, 308): same DMA-ramp coverage as (256, 256) but
        # the remainder packs into one fewer tile (64 fewer matmuls)
        for s in (TILE0, 308):
            tiles.append((off, s))
            off += s
            cap -= s
    tail = 0
    if last_small and cap > 4 * TILE0:
        tail = TILE0
        cap -= TILE0
    n = -(-cap // TOK_TILE)
    base, rem = divmod(cap, n)
    for i in range(n):
        t = base + (1 if i < rem else 0)
        tiles.append((off, t))
        off += t
    if tail:
        tiles.append((off, tail))
    return tiles


def build_moe_nc(n_toks: tuple, act: str = "Gelu"):
    """Build + compile the per-core Bass program.

    n_toks[e] = number of tokens routed to expert e (same on all cores;
    every core sees every token, sliced along F)."""
    import concourse.mybir as mybir
    import concourse.tile as tile
    from concourse import bacc

    dt = mybir.dt
    GELU = getattr(mybir.ActivationFunctionType, act)
    IDENT = mybir.ActivationFunctionType.Identity

    ntok = int(sum(n_toks))

    nc = bacc.Bacc("TRN2", target_bir_lowering=False, debug=False)

    xt_d = nc.dram_tensor("xt", [128, KC, ntok], dt.bfloat16, kind="ExternalInput")
    w1_d = nc.dram_tensor("w1l", [128, E, KC, FL], dt.bfloat16, kind="ExternalInput")
    b1_d = nc.dram_tensor("b1l", [128, E * KFL], dt.float32, kind="ExternalInput")
    w2_d = nc.dram_tensor("w2l", [128, E, KFL, C], dt.bfloat16, kind="ExternalInput")
    yt_d = nc.dram_tensor("yt", [128, KC, ntok], dt.bfloat16, kind="ExternalOutput")

    # global tile list: (expert, global token offset, size)
    seg_off = [0]
    for e in range(E):
        seg_off.append(seg_off[-1] + int(n_toks[e]))
    all_tiles = []
    for e in range(E):
        if n_toks[e] == 0:
            continue
        for off, tsz in _token_tiles(int(n_toks[e]), first_small=(e == 0)):
            all_tiles.append((e, seg_off[e] + off, tsz))
    n_tiles = len(all_tiles)
    # first tile index of each expert (where to JIT-load weights)
    first_tile_of = {}
    for i, (e, _, _) in enumerate(all_tiles):
        first_tile_of.setdefault(e, i)

    with tile.TileContext(nc) as tc:
        with (
            tc.tile_pool(name="wpool", bufs=1) as wpool,
            tc.tile_pool(name="xpool", bufs=3) as xpool,
            tc.tile_pool(name="hpool", bufs=2) as hpool,
            tc.tile_pool(name="ypool", bufs=2) as ypool,
            tc.tile_pool(name="pp", bufs=8, space="PSUM") as pp,
        ):
            # all weights load in half-expert chunks (~0.5 MB): small
            # enough that an xt trigger queued behind one is never late,
            # big enough to keep the HWDGE instruction count trivial
            w1_s: list = [[None, None] for _ in range(E)]
            w2_s: list = [[None, None] for _ in range(E)]
            HKC, HKF = KC // 2, KFL // 2

            def load_w_chunk(e, j):
                if j < 2:  # w1 half j
                    w = wpool.tile([128, HKC, FL], dt.bfloat16,
                                   tag=f"w1_{e}_{j}", name=f"w1_{e}_{j}")
                    nc.sync.dma_start(w[:], w1_d[:, e, j * HKC : (j + 1) * HKC, :])
                    w1_s[e][j] = w
                else:      # w2 half j-2
                    h = j - 2
                    w = wpool.tile([128, HKF, C], dt.bfloat16,
                                   tag=f"w2_{e}_{h}", name=f"w2_{e}_{h}")
                    nc.sync.dma_start(w[:], w2_d[:, e, h * HKF : (h + 1) * HKF, :])
                    w2_s[e][h] = w

            def load_xt(t):
                _, goff, tsz = all_tiles[t]
                xk = xpool.tile([128, KC, tsz], dt.bfloat16, tag="xt")
                nc.sync.dma_start(xk[:], xt_d[:, :, goff : goff + tsz])
                return xk

            # --- prefetch, in consumption order; everything beyond
            # expert 1 streams just-in-time during the tile loop.
            # Tile 0's xt splits into kc-halves so the very first matmul
            # chain only waits on w1-half0 + xt-half0 (~1 MB).
            load_w_chunk(0, 0)
            tsz0 = all_tiles[0][2]
            xt0_h = []
            for h in range(2):
                xh = xpool.tile([128, HKC, tsz0], dt.bfloat16,
                                tag=f"xt0_{h}", name=f"xt0_{h}")
                nc.sync.dma_start(
                    xh[:], xt_d[:, h * HKC : (h + 1) * HKC, :tsz0]
                )
                xt0_h.append(xh)
                if h == 0:
                    load_w_chunk(0, 1)
            xt_tiles: dict[int, object] = {0: xt0_h}
            b1_s = wpool.tile([128, E * KFL], dt.float32, tag="b1")
            nc.sync.dma_start(b1_s[:], b1_d[:])
            if 1 < n_tiles:
                xt_tiles[1] = load_xt(1)
            load_w_chunk(0, 2)
            if 2 < n_tiles:
                xt_tiles[2] = load_xt(2)
            load_w_chunk(0, 3)
            if E > 1:
                for j in range(4):
                    load_w_chunk(1, j)

            def w1_ap(e, kc, mf):
                h, r = divmod(kc, HKC)
                return w1_s[e][h][:, r, mf * 128 : (mf + 1) * 128]

            def w2_ap(e, kf, mc):
                h, r = divmod(kf, HKF)
                return w2_s[e][h][:, r, mc * 128 : (mc + 1) * 128]

            ht_tiles: dict[int, object] = {}

            def emit_L1(t):
                e, _, tsz = all_tiles[t]
                xt_s = xt_tiles.pop(t)
                ht_s = hpool.tile([128, KFL, tsz], dt.bfloat16, tag="ht")
                ht_tiles[t] = ht_s
                if t == 0:
                    # kc-half-outer accumulation over half-split w1/xt:
                    # starts on the first w1/xt half-DMAs while the
                    # second halves stream in
                    ps_w = [
                        pp.tile([128, tsz], dt.float32, tag="ps", name=f"ps0_{i}")
                        for i in range(KFL)
                    ]
                    for kc in range(KC):
                        for mf in range(KFL):
                            nc.tensor.matmul(
                                ps_w[mf][:],
                                w1_ap(e, kc, mf),
                                xt_s[kc // HKC][:, kc % HKC, :],
                                start=(kc == 0), stop=(kc == KC - 1),
                            )
                    for mf in range(KFL):
                        nc.scalar.activation(
                            ht_s[:, mf, :], ps_w[mf][:], GELU,
                            bias=b1_s[:, e * KFL + mf : e * KFL + mf + 1],
                        )
                    return
                for mf in range(KFL):
                    ps = pp.tile([128, tsz], dt.float32, tag="ps")
                    for kc in range(KC):
                        nc.tensor.matmul(
                            ps[:], w1_ap(e, kc, mf), xt_s[:, kc, :],
                            start=(kc == 0), stop=(kc == KC - 1),
                        )
                    nc.scalar.activation(
                        ht_s[:, mf, :], ps[:], GELU,
                        bias=b1_s[:, e * KFL + mf : e * KFL + mf + 1],
                    )

            def emit_L2(t):
                e, goff, tsz = all_tiles[t]
                last = t == n_tiles - 1
                ht_s = ht_tiles.pop(t)
                y_s = ypool.tile([128, KC, tsz], dt.bfloat16, tag="y")
                for mc in range(KC):
                    ps2 = pp.tile([128, tsz], dt.float32, tag="ps")
                    for kf in range(KFL):
                        nc.tensor.matmul(
                            ps2[:],
                            w2_ap(e, kf, mc),
                            ht_s[:, kf, :],
                            start=(kf == 0), stop=(kf == KFL - 1),
                        )
                    nc.scalar.activation(y_s[:, mc, :], ps2[:], IDENT)
                    if last and mc == KC - 3:
                        # final tile drains in a 6/2 split on the sync
                        # engine (idle by then): the 6-block DMA fires
                        # two chains before the end and overlaps them,
                        # leaving only one trigger + a 0.25 MB transfer
                        # after the last cast — short AND deterministic
                        # tail (8 per-mc triggers cost 8 serial ~625ns
                        # HWDGE slots, which sometimes failed to overlap)
                        nc.sync.dma_start(
                            yt_d[:, : KC - 2, goff : goff + tsz],
                            y_s[:, : KC - 2, :],
                        )
                if last:
                    nc.sync.dma_start(
                        yt_d[:, KC - 2 :, goff : goff + tsz],
                        y_s[:, KC - 2 :, :],
                    )
                else:
                    # y goes out via the scalar engine's queue so output
                    # DMAs never delay xt/weight triggers on sync (NOTE:
                    # gpsimd DMA would work too but Q7 activity downclocks
                    # the PE by ~20% — measured)
                    nc.scalar.dma_start(yt_d[:, :, goff : goff + tsz], y_s[:])

            # --- software-pipelined main loop: L1 runs one tile ahead.
            # Expert e+1's four weight chunks are spread over the first
            # four tiles of expert e's segment, each emitted AFTER that
            # tile's xt trigger so xt is never queued behind a weight.
            chunks_done = [0] * E
            chunks_done[0] = 4
            if E > 1:
                chunks_done[1] = 4

            emit_L1(0)
            for t in range(n_tiles):
                if t + 1 < n_tiles:
                    e_next = all_tiles[t + 1][0]
                    if t + 1 not in xt_tiles:
                        xt_tiles[t + 1] = load_xt(t + 1)
                    # safety: e_next's own missing chunks load right now
                    while chunks_done[e_next] < 4:
                        load_w_chunk(e_next, chunks_done[e_next])
                        chunks_done[e_next] += 1
                    tgt = e_next + 1
                    if tgt < E:
                        k = t + 1 - first_tile_of[e_next]
                        while chunks_done[tgt] <= min(k, 3):
                            load_w_chunk(tgt, chunks_done[tgt])
                            chunks_done[tgt] += 1
                    emit_L1(t + 1)
                emit_L2(t)

    nc.compile()
    return nc


def _route(x_flat, gate_w, gate_b):
    """Replicates reference gating: softmax -> top-2 -> renormalize."""
    logits = x_flat @ gate_w + gate_b  # [N, E] f32
    m = logits.max(-1, keepdims=True)
    p = np.exp(logits - m)
    p /= p.sum(-1, keepdims=True)
    # jax.lax.top_k: descending, ties -> lower index. Stable argsort matches.
    order = np.argsort(-p, axis=1, kind="stable")[:, :TOPK]  # [N, 2]
    top = np.take_along_axis(p, order, axis=1)
    wts = top / top.sum(-1, keepdims=True)
    return order, wts.astype(np.float32)


def run_moe(inputs: dict, trace: bool = False):
    """Returns (full_output [B,T,C] f32, BassKernelResults)."""
    from concourse.bass_utils import run_bass_kernel_spmd

    x = np.asarray(inputs["x"], dtype=np.float32)
    gate_w = np.asarray(inputs["gate_w"], dtype=np.float32)
    gate_b = np.asarray(inputs["gate_b"], dtype=np.float32)
    w1 = np.asarray(inputs["w1"], dtype=np.float32)
    b1 = np.asarray(inputs["b1"], dtype=np.float32)
    w2 = np.asarray(inputs["w2"], dtype=np.float32)
    b2 = np.asarray(inputs["b2"], dtype=np.float32)

    xf = x.reshape(-1, C)
    order, wts = _route(xf, gate_w, gate_b)

    idx = []
    comb = []
    for e in range(E):
        mask = order == e  # [N, 2]
        rows = np.nonzero(mask.any(axis=1))[0]
        idx.append(rows)
        comb.append((wts[rows] * mask[rows]).sum(axis=1).astype(np.float32))
    n_toks = tuple(len(r) for r in idx)
    ntok = int(sum(n_toks))

    if n_toks not in _nc_cache:
        _nc_cache[n_toks] = build_moe_nc(n_toks)
    nc = _nc_cache[n_toks]

    # xt: all segments concatenated, transposed — identical on every core
    xcat = np.empty((ntok, C), dtype=np.float32)
    off = 0
    for e in range(E):
        xcat[off : off + n_toks[e]] = xf[idx[e]]
        off += n_toks[e]
    # [ntok, C] -> [128, KC, ntok]  (partition-major for single-DMA tiles)
    xt = np.ascontiguousarray(
        xcat.T.reshape(KC, 128, ntok).transpose(1, 0, 2).astype(_BF16)
    )

    w1b = w1.astype(_BF16)  # [E, C, F]
    w2b = w2.astype(_BF16)  # [E, F, C]

    in_maps = []
    for c in range(N_CORES):
        lo, hi = c * FL, (c + 1) * FL
        w1l = np.ascontiguousarray(
            w1b[:, :, lo:hi].reshape(E, KC, 128, FL).transpose(2, 0, 1, 3)
        )
        w2l = np.ascontiguousarray(
            w2b[:, lo:hi, :].reshape(E, KFL, 128, C).transpose(2, 0, 1, 3)
        )
        b1l = np.ascontiguousarray(
            b1[:, lo:hi].reshape(E * KFL, 128).T.astype(np.float32)
        )
        in_maps.append({"xt": xt, "w1l": w1l, "b1l": b1l, "w2l": w2l})

    res = run_bass_kernel_spmd(nc, in_maps, list(range(N_CORES)), trace=trace)

    # host combine: sum the 8 partial y's, add b2, apply combine weights
    ysum = np.zeros((128, KC, ntok), dtype=np.float32)
    for c in range(N_CORES):
        ysum += res.results[c]["yt"]
    ysum = ysum.transpose(1, 0, 2).reshape(C, ntok)

    out = np.zeros_like(xf)
    off = 0
    for e in range(E):
        n_e = n_toks[e]
        if n_e == 0:
            continue
        y = ysum[:, off : off + n_e].T + b2[e]  # [n_e, C]
        out[idx[e]] += comb[e][:, None] * y
        off += n_e
    return out.reshape(B, T, C), res


def kernel(x, gate_w, gate_b, w1, b1, w2, b2):
    out, _ = run_moe(
        {
            "x": x,
            "gate_w": gate_w,
            "gate_b": gate_b,
            "w1": w1,
            "b1": b1,
            "w2": w2,
            "b2": b2,
        }
    )
    return out

